# revision 6
# baseline (speedup 1.0000x reference)
"""Layer-pipelined Trainium2 Bass kernel for the 2-layer tanh RNN + FC.

Topology: 4 core pairs {i, i+4}, each owning 16 batch samples. Low
cores run layer 0, high cores layer 1, with a one-chunk pipeline lag.
All 8 cores execute the SAME instruction stream (SPMD, no branching);
role differences are carried entirely by per-core input data:

  - uin:   low cores = x chunks (D zero-padded 256->512), high = zeros
  - usb  = uin[chunk s] + msel * AllGather(prev chunk of partner)
           (msel = 0 on low cores, 1 on high cores)
  - whh/wih/brow = the core's layer weights (wih0 zero-padded to 512)
  - maskvec[s] multiplies the hidden state carried across the chunk
    boundary (0 exactly where each role's recurrence must restart)

Per step each core runs only ONE layer's chain: 4 fp16 matmuls,
1 ScalarE copy, 1 DMA-xbar transpose, 1 DVE add, 1 ScalarE tanh --
roughly half the fused-two-layer instruction stream, which is what
the backend's per-instruction cost model rewards.
"""

import sys

if "/opt/trn_rl_repo" not in sys.path:
    sys.path.insert(0, "/opt/trn_rl_repo")

import numpy as np

import concourse.bacc as bacc
import concourse.mybir as mybir
import concourse.tile as tile
from concourse import bass_utils

F16 = mybir.dt.float16
F32 = mybir.dt.float32
AF = mybir.ActivationFunctionType

N_CORES = 8
NPAIR = 4
B, T, D, H, O = 64, 1024, 256, 512, 256
BCp = B // NPAIR  # batch per pair: 16

CH = 128  # timesteps per chunk
KH = H // 128  # 4 hidden chunks
KU = 4  # unified input-contraction chunks (D padded to 512)
MO = O // 128  # 2
NB = CH * BCp  # 2048 (t, b) columns per chunk
GROUPS = [[0, 4], [1, 5], [2, 6], [3, 7]]


def build(T=T, CH=CH, reps=1):
    NCH = T // CH
    NSLOT = NCH + 1
    NS = NB // 512  # 512-col moving splits per chunk: 4

    nc = bacc.Bacc("TRN2", target_bir_lowering=False, debug=False,
                   num_devices=N_CORES)

    xu_d = nc.dram_tensor("xu", [2 * 128, NSLOT * NB], F16,
                          kind="ExternalInput")
    wih_d = nc.dram_tensor("wihT", [512, H], F16, kind="ExternalInput")
    whh_d = nc.dram_tensor("whhT", [H, H], F16, kind="ExternalInput")
    wfc_d = nc.dram_tensor("wfcT", [H, O], F16, kind="ExternalInput")
    brow_d = nc.dram_tensor("brow", [128, KH], F32, kind="ExternalInput")
    bfc_d = nc.dram_tensor("bfc", [128, MO], F32, kind="ExternalInput")
    msel_d = nc.dram_tensor("msel", [128, 1], F32, kind="ExternalInput")
    mvec_d = nc.dram_tensor("mvec", [128, NSLOT + 1], F32,
                            kind="ExternalInput")
    ccin_d = nc.dram_tensor("ccin", [KH * 128, NB], F16, kind="Internal")
    ccout_d = nc.dram_tensor("ccout", [2 * KH * 128, NB], F16,
                             kind="Internal")
    out_d = nc.dram_tensor("out", [O, BCp], F32, kind="ExternalOutput")

    with tile.TileContext(nc) as tc:
        with (
            tc.tile_pool(name="wpool", bufs=1) as wpool,
            tc.tile_pool(name="upool", bufs=1) as upool,
            tc.tile_pool(name="chunks", bufs=2) as chpool,
            tc.tile_pool(name="hprevs", bufs=2) as hpool,
            tc.tile_pool(name="state", bufs=4) as spool,
            tc.tile_pool(name="psx", bufs=2, space="PSUM") as psx_pool,
            tc.tile_pool(name="psz", bufs=2, space="PSUM") as psz_pool,
        ):
            # ---- weight/bias preload ----
            wih = wpool.tile([128, KU * KH * 128], F16, name="wih")
            for k in range(KU):
                nc.sync.dma_start(wih[:, k * KH * 128:(k + 1) * KH * 128],
                                  wih_d[k * 128:(k + 1) * 128, :])
            whh = wpool.tile([128, KH * 512], F16, name="whh")
            for k in range(KH):
                nc.sync.dma_start(whh[:, k * 512:(k + 1) * 512],
                                  whh_d[k * 128:(k + 1) * 128, :])
            wfc = wpool.tile([128, KH * MO * 128], F16, name="wfc")
            for k in range(KH):
                nc.sync.dma_start(wfc[:, k * MO * 128:(k + 1) * MO * 128],
                                  wfc_d[k * 128:(k + 1) * 128, :])
            brow = wpool.tile([128, KH], F32, name="brow")
            nc.sync.dma_start(brow[:], brow_d[:])
            bfc = wpool.tile([128, MO], F32, name="bfcs")
            nc.sync.dma_start(bfc[:], bfc_d[:])
            msel = wpool.tile([128, 1], F32, name="msel")
            nc.sync.dma_start(msel[:], msel_d[:])
            mvec = wpool.tile([128, NSLOT + 1], F32, name="mvec")
            nc.sync.dma_start(mvec[:], mvec_d[:])
            hz = wpool.tile([128, KH, BCp], F16, name="hzero")
            nc.vector.memset(hz[:], 0.0)
            zero2k = wpool.tile([128, NB], F16, name="zero2k")
            nc.vector.memset(zero2k[:], 0.0)

            def wsl(wt, k, m, mt):
                c = (k * mt + m) * 128
                return wt[:, c:c + 128]

            for _rep in range(reps):
                # ccout must be zero before slot 0 reads it
                for r in range(2 * KH):
                    nc.sync.dma_start(ccout_d[r * 128:(r + 1) * 128, :],
                                      zero2k[:])
                hprev = hz
                hct = None
                for s in range(NSLOT):
                    # ---- uniform input select:
                    #   usb = xu[chunk s] (D-padded) + msel * ccout[slot 0]
                    usb_x = upool.tile([128, KU, NB], F16, name="usbx")
                    for k in range(2):
                        nc.sync.dma_start(
                            usb_x[:, k, :],
                            xu_d[k * 128:(k + 1) * 128,
                                 s * NB:(s + 1) * NB])
                    nc.vector.memset(usb_x[:, 2:4, :], 0.0)
                    usb_cc = upool.tile([128, KU, NB], F16, name="usbcc")
                    for k in range(KU):
                        nc.sync.dma_start(usb_cc[:, k, :],
                                          ccout_d[k * 128:(k + 1) * 128, :])
                    usb_m = upool.tile([128, KU, NB], F16, name="usbm")
                    nc.vector.tensor_scalar_mul(usb_m[:], usb_cc[:],
                                                msel[:, 0:1])
                    usb = upool.tile([128, KU, NB], F16, name="usb")
                    nc.vector.tensor_add(usb[:], usb_x[:], usb_m[:])

                    # ---- projection: xwc[p, t, m, b] ----
                    xwc = chpool.tile([128, CH, KH, BCp], F16, name="xwc")
                    for m in range(KH):
                        for n in range(NS):
                            ps = psx_pool.tile([128, 512], F32, name="psxt")
                            for k in range(KU):
                                nc.tensor.matmul(
                                    ps[:],
                                    wsl(wih, k, m, KH),
                                    usb[:, k, n * 512:(n + 1) * 512],
                                    start=(k == 0),
                                    stop=(k == KU - 1),
                                )
                            tpn = 512 // BCp  # timesteps per split: 32
                            nc.scalar.activation(
                                xwc[:, n * tpn:(n + 1) * tpn, m, :],
                                ps[:].rearrange("p (t b) -> p t b", b=BCp),
                                AF.Identity,
                                bias=brow[:, m:m + 1],
                            )

                    # ---- recurrence (one layer, 128 steps) ----
                    hct = chpool.tile([128, CH, KH, BCp], F16, name="hct")
                    for tl in range(CH):
                        hsrc = hprev if tl == 0 else hct[:, tl - 1]
                        psz = psz_pool.tile([BCp, 512], F32, name="z")
                        for j in range(KH):
                            nc.tensor.matmul(
                                psz[:],
                                hsrc[:, j, :],
                                whh[:, j * 512:(j + 1) * 512],
                                start=(j == 0),
                                stop=(j == KH - 1),
                            )
                        zsb = spool.tile([BCp, 512], F16, name="zsb")
                        nc.scalar.activation(zsb[:], psz[:], AF.Identity)
                        zt = spool.tile([128, KH, BCp], F16, name="zt")
                        nc.sync.dma_start(zt[:], zsb[:], transpose=True)
                        zpre = spool.tile([128, KH, BCp], F32, name="zpre")
                        nc.vector.tensor_add(zpre[:], zt[:], xwc[:, tl])
                        nc.scalar.activation(hct[:, tl], zpre[:], AF.Tanh)

                    # ---- handoff: send hct, gather within pair ----
                    if s < NSLOT - 1:
                        for j in range(KH):
                            nc.sync.dma_start(
                                ccin_d[j * 128:(j + 1) * 128, :],
                                hct[:, :, j, :],
                            )
                        nc.gpsimd.collective_compute(
                            "AllGather",
                            mybir.AluOpType.bypass,
                            ins=[ccin_d[:]],
                            outs=[ccout_d[:]],
                            replica_groups=GROUPS,
                        )
                        # ---- boundary state for the next chunk ----
                        hprev = hpool.tile([128, KH, BCp], F16, name="hprev")
                        nc.vector.tensor_scalar_mul(
                            hprev[:], hct[:, CH - 1], mvec[:, s + 1:s + 2])

            # ---- final FC on hct[:, CH-1] ----
            h1f = hct[:, CH - 1]
            psf = psx_pool.tile([128, MO * BCp], F32, name="psxt")
            for m in range(MO):
                for k in range(KH):
                    nc.tensor.matmul(
                        psf[:, m * BCp:(m + 1) * BCp],
                        wsl(wfc, k, m, MO),
                        h1f[:, k, :],
                        start=(k == 0),
                        stop=(k == KH - 1),
                    )
            outs = spool.tile([128, MO * BCp], F32, name="outs")
            for m in range(MO):
                nc.scalar.activation(
                    outs[:, m * BCp:(m + 1) * BCp],
                    psf[:, m * BCp:(m + 1) * BCp],
                    AF.Identity,
                    bias=bfc[:, m:m + 1],
                )
            for m in range(MO):
                nc.sync.dma_start(out_d[m * 128:(m + 1) * 128, :],
                                  outs[:, m * BCp:(m + 1) * BCp])

    nc.compile()
    return nc


def make_in_maps(inputs, T=T):
    x = np.asarray(inputs["x"], np.float32)
    NCH = T // CH
    NSLOT = NCH + 1

    def t16(a):
        return np.asarray(a, np.float32).T.astype(np.float16)

    def padT(a):  # [H, d] -> [512, H] transposed + zero-padded
        t = t16(a)
        out = np.zeros((512, t.shape[1]), np.float16)
        out[:t.shape[0]] = t
        return out

    def browf(bi, bh):
        return np.ascontiguousarray(
            (np.asarray(bi, np.float32) + np.asarray(bh, np.float32))
            .reshape(KH, 128).T)

    wfcT = np.ascontiguousarray(t16(inputs["W_fc"]))
    bfc = np.ascontiguousarray(
        np.asarray(inputs["b_fc"], np.float32).reshape(MO, 128).T)
    shared = {"wfcT": wfcT, "bfc": bfc}

    l0 = {
        "wihT": np.ascontiguousarray(padT(inputs["W_ih0"])),
        "whhT": np.ascontiguousarray(t16(inputs["W_hh0"])),
        "brow": browf(inputs["b_ih0"], inputs["b_hh0"]),
        "msel": np.zeros((128, 1), np.float32),
        "mvec": np.broadcast_to(
            np.r_[0.0, np.ones(NSLOT, np.float32)], (128, NSLOT + 1)).copy(),
    }
    l1 = {
        "wihT": np.ascontiguousarray(padT(inputs["W_ih1"])),
        "whhT": np.ascontiguousarray(t16(inputs["W_hh1"])),
        "brow": browf(inputs["b_ih1"], inputs["b_hh1"]),
        "msel": np.ones((128, 1), np.float32),
        "mvec": np.broadcast_to(
            np.r_[0.0, 0.0, np.ones(NSLOT - 1, np.float32)],
            (128, NSLOT + 1)).copy(),
    }

    zero_xu = np.zeros((2 * 128, NSLOT * NB), np.float16)
    in_maps = [None] * N_CORES
    for i in range(NPAIR):
        xs = x[i * BCp:(i + 1) * BCp, :T]  # [16, T, D]
        # xu[k, p, s*NB + t*BCp + b] = x[b, s*CH+t, k*128+p]
        xu = np.zeros((2, 128, NSLOT, CH, BCp), np.float16)
        xu[:, :, :NCH] = (xs.transpose(2, 1, 0).astype(np.float16)
                          .reshape(2, 128, NCH, CH, BCp))
        xu = np.ascontiguousarray(xu.reshape(2 * 128, NSLOT * NB))
        in_maps[i] = {"xu": xu, **l0, **shared}
        in_maps[i + NPAIR] = {"xu": zero_xu, **l1, **shared}
    return in_maps


def assemble_out(results):
    out = np.empty((B, O), np.float32)
    for i in range(NPAIR):
        out[i * BCp:(i + 1) * BCp] = results[NPAIR + i]["out"].T
    return out


_NC_CACHE = {}


def kernel(**inputs) -> np.ndarray:
    if "nc" not in _NC_CACHE:
        _NC_CACHE["nc"] = build()
    nc = _NC_CACHE["nc"]
    in_maps = make_in_maps(inputs)
    res = bass_utils.run_bass_kernel_spmd(nc, in_maps, list(range(N_CORES)))
    return assemble_out(res.results)


# revision 8
# speedup vs baseline: 1.5949x; 1.5949x over previous
"""Layer-pipelined Trainium2 Bass kernel for the 2-layer tanh RNN + FC.

Topology: 4 core pairs {i, i+4}, each owning 16 batch samples. Low
cores run layer 0, high cores layer 1, with a one-chunk pipeline lag.
All 8 cores execute the SAME instruction stream (SPMD, no branching);
role differences are carried entirely by per-core input data:

  - uin:   low cores = x chunks (D zero-padded 256->512), high = zeros
  - usb  = uin[chunk s] + msel * AllGather(prev chunk of partner)
           (msel = 0 on low cores, 1 on high cores)
  - whh/wih/brow = the core's layer weights (wih0 zero-padded to 512)
  - maskvec[s] multiplies the hidden state carried across the chunk
    boundary (0 exactly where each role's recurrence must restart)

Per step each core runs only ONE layer's chain: 4 fp32 matmuls
(fp32 operands make each matmul a single self-loading PE instruction
-- fp16 pays a separate LDWEIGHTS per matmul), 1 ScalarE copy, 1
DMA-xbar transpose, 1 DVE add, 1 ScalarE tanh -- roughly half the
fused-two-layer instruction stream. The chunk handoff converts the
fp32 hidden chunk to fp16 once (bulk DVE copy) so the AllGather and
the projection inputs stay 2-byte.
"""

import sys

if "/opt/trn_rl_repo" not in sys.path:
    sys.path.insert(0, "/opt/trn_rl_repo")

import numpy as np

import concourse.bacc as bacc
import concourse.mybir as mybir
import concourse.tile as tile
from concourse import bass_utils

F16 = mybir.dt.float16
F32 = mybir.dt.float32
AF = mybir.ActivationFunctionType

N_CORES = 8
NPAIR = 4
B, T, D, H, O = 64, 1024, 256, 512, 256
BCp = B // NPAIR  # batch per pair: 16

CH = 128  # timesteps per chunk
KH = H // 128  # 4 hidden chunks
KU = 4  # unified input-contraction chunks (D padded to 512)
MO = O // 128  # 2
NB = CH * BCp  # 2048 (t, b) columns per chunk
GROUPS = [[0, 4], [1, 5], [2, 6], [3, 7]]


def build(T=T, CH=CH, reps=1):
    NCH = T // CH
    NSLOT = NCH + 1
    NS = NB // 512  # 512-col moving splits per chunk: 4

    nc = bacc.Bacc("TRN2", target_bir_lowering=False, debug=False,
                   num_devices=N_CORES)

    xu_d = nc.dram_tensor("xu", [2 * 128, NCH * NB], F16,
                          kind="ExternalInput")
    wih_d = nc.dram_tensor("wihT", [512, H], F32, kind="ExternalInput")
    whh_d = nc.dram_tensor("whhT", [H, H], F32, kind="ExternalInput")
    wfc_d = nc.dram_tensor("wfcT", [H, O], F32, kind="ExternalInput")
    brow_d = nc.dram_tensor("brow", [128, KH], F32, kind="ExternalInput")
    bfc_d = nc.dram_tensor("bfc", [128, MO], F32, kind="ExternalInput")
    msel_d = nc.dram_tensor("msel", [128, 1], F32, kind="ExternalInput")
    mvec_d = nc.dram_tensor("mvec", [128, NSLOT + 1], F32,
                            kind="ExternalInput")
    ccin_d = nc.dram_tensor("ccin", [KH * 128, NB], F16, kind="Internal")
    ccout_d = nc.dram_tensor("ccout", [2 * KH * 128, NB], F16,
                             kind="Internal")
    out_d = nc.dram_tensor("out", [O, BCp], F32, kind="ExternalOutput")

    with tile.TileContext(nc) as tc:
        with (
            tc.tile_pool(name="wpool", bufs=1) as wpool,
            tc.tile_pool(name="upool", bufs=1) as upool,
            tc.tile_pool(name="chunks", bufs=2) as chpool,
            tc.tile_pool(name="hprevs", bufs=2) as hpool,
            tc.tile_pool(name="hcts", bufs=1) as hctpool,
            tc.tile_pool(name="state", bufs=4) as spool,
            tc.tile_pool(name="psx", bufs=2, space="PSUM") as psx_pool,
            tc.tile_pool(name="psz", bufs=2, space="PSUM") as psz_pool,
        ):
            # ---- weight/bias preload ----
            wih = wpool.tile([128, KU * KH * 128], F32, name="wih")
            for k in range(KU):
                nc.sync.dma_start(wih[:, k * KH * 128:(k + 1) * KH * 128],
                                  wih_d[k * 128:(k + 1) * 128, :])
            whh = wpool.tile([128, KH * 512], F32, name="whh")
            for k in range(KH):
                nc.sync.dma_start(whh[:, k * 512:(k + 1) * 512],
                                  whh_d[k * 128:(k + 1) * 128, :])
            wfc = wpool.tile([128, KH * MO * 128], F32, name="wfc")
            for k in range(KH):
                nc.sync.dma_start(wfc[:, k * MO * 128:(k + 1) * MO * 128],
                                  wfc_d[k * 128:(k + 1) * 128, :])
            brow = wpool.tile([128, KH], F32, name="brow")
            nc.sync.dma_start(brow[:], brow_d[:])
            bfc = wpool.tile([128, MO], F32, name="bfcs")
            nc.sync.dma_start(bfc[:], bfc_d[:])
            msel = wpool.tile([128, 1], F32, name="msel")
            nc.sync.dma_start(msel[:], msel_d[:])
            mvec = wpool.tile([128, NSLOT + 1], F32, name="mvec")
            nc.sync.dma_start(mvec[:], mvec_d[:])
            hz = wpool.tile([128, KH, BCp], F32, name="hzero")
            nc.vector.memset(hz[:], 0.0)
            zero2k = wpool.tile([128, NB], F16, name="zero2k")
            nc.vector.memset(zero2k[:], 0.0)

            def wsl(wt, k, m, mt):
                c = (k * mt + m) * 128
                return wt[:, c:c + 128]

            for _rep in range(reps):
                # ccout must be zero before slot 0 reads it
                for r in range(2 * KH):
                    nc.sync.dma_start(ccout_d[r * 128:(r + 1) * 128, :],
                                      zero2k[:])
                hprev = hz
                hct = None
                for s in range(NSLOT):
                    # ---- uniform input select:
                    #   usb = xu[chunk s] (D-padded) + msel * ccout[slot 0]
                    usb_x = upool.tile([128, KU, NB], F16, name="usbx")
                    sx = min(s, NCH - 1)  # slot NCH reuses a stale chunk
                    for k in range(2):
                        nc.sync.dma_start(
                            usb_x[:, k, :],
                            xu_d[k * 128:(k + 1) * 128,
                                 sx * NB:(sx + 1) * NB])
                    nc.vector.memset(usb_x[:, 2:4, :], 0.0)
                    usb_cc = upool.tile([128, KU, NB], F16, name="usbcc")
                    for k in range(KU):
                        nc.sync.dma_start(usb_cc[:, k, :],
                                          ccout_d[k * 128:(k + 1) * 128, :])
                    usb_m = upool.tile([128, KU, NB], F16, name="usbm")
                    nc.vector.tensor_scalar_mul(usb_m[:], usb_cc[:],
                                                msel[:, 0:1])
                    usb = upool.tile([128, KU, NB], F32, name="usb")
                    nc.vector.tensor_add(usb[:], usb_x[:], usb_m[:])

                    # ---- projection: xwc[p, t, m, b] ----
                    xwc = chpool.tile([128, CH, KH, BCp], F16, name="xwc")
                    for m in range(KH):
                        for h2 in range(NS // 2):
                            ps = psx_pool.tile([128, 1024], F32, name="psxt")
                            for n2 in range(2):
                                n = h2 * 2 + n2
                                for k in range(KU):
                                    nc.tensor.matmul(
                                        ps[:, n2 * 512:(n2 + 1) * 512],
                                        wsl(wih, k, m, KH),
                                        usb[:, k, n * 512:(n + 1) * 512],
                                        start=(k == 0),
                                        stop=(k == KU - 1),
                                    )
                            tpn = 1024 // BCp  # timesteps per evac: 64
                            nc.scalar.activation(
                                xwc[:, h2 * tpn:(h2 + 1) * tpn, m, :],
                                ps[:].rearrange("p (t b) -> p t b", b=BCp),
                                AF.Identity,
                                bias=brow[:, m:m + 1],
                            )

                    # ---- recurrence (one layer, 128 steps) ----
                    hct = hctpool.tile([128, CH, KH, BCp], F32, name="hct")
                    for tl in range(CH):
                        hsrc = hprev if tl == 0 else hct[:, tl - 1]
                        psz = psz_pool.tile([BCp, 512], F32, name="z")
                        for j in range(KH):
                            nc.tensor.matmul(
                                psz[:],
                                hsrc[:, j, :],
                                whh[:, j * 512:(j + 1) * 512],
                                start=(j == 0),
                                stop=(j == KH - 1),
                            )
                        zsb = spool.tile([BCp, 512], F16, name="zsb")
                        nc.scalar.activation(zsb[:], psz[:], AF.Identity)
                        zt = spool.tile([128, KH, BCp], F16, name="zt")
                        nc.sync.dma_start(zt[:], zsb[:], transpose=True)
                        zpre = spool.tile([128, KH, BCp], F32, name="zpre")
                        nc.vector.tensor_add(zpre[:], zt[:], xwc[:, tl])
                        nc.scalar.activation(hct[:, tl], zpre[:], AF.Tanh)

                    # ---- handoff: send hct (as fp16), gather in pair ----
                    if s < NSLOT - 1:
                        hct16 = hctpool.tile([128, CH, KH, BCp], F16,
                                             name="hct16")
                        nc.vector.tensor_copy(hct16[:], hct[:])
                        for j in range(KH):
                            nc.sync.dma_start(
                                ccin_d[j * 128:(j + 1) * 128, :],
                                hct16[:, :, j, :],
                            )
                        nc.gpsimd.collective_compute(
                            "AllGather",
                            mybir.AluOpType.bypass,
                            ins=[ccin_d[:]],
                            outs=[ccout_d[:]],
                            replica_groups=GROUPS,
                        )
                        # ---- boundary state for the next chunk ----
                        hprev = hpool.tile([128, KH, BCp], F32, name="hprev")
                        nc.vector.tensor_scalar_mul(
                            hprev[:], hct[:, CH - 1], mvec[:, s + 1:s + 2])

            # ---- final FC on hct[:, CH-1] ----
            h1f = hct[:, CH - 1]
            psf = psx_pool.tile([128, MO * BCp], F32, name="psxt")
            for m in range(MO):
                for k in range(KH):
                    nc.tensor.matmul(
                        psf[:, m * BCp:(m + 1) * BCp],
                        wsl(wfc, k, m, MO),
                        h1f[:, k, :],
                        start=(k == 0),
                        stop=(k == KH - 1),
                    )
            outs = spool.tile([128, MO * BCp], F32, name="outs")
            for m in range(MO):
                nc.scalar.activation(
                    outs[:, m * BCp:(m + 1) * BCp],
                    psf[:, m * BCp:(m + 1) * BCp],
                    AF.Identity,
                    bias=bfc[:, m:m + 1],
                )
            for m in range(MO):
                nc.sync.dma_start(out_d[m * 128:(m + 1) * 128, :],
                                  outs[:, m * BCp:(m + 1) * BCp])

    nc.compile()
    return nc


def make_in_maps(inputs, T=T):
    x = np.asarray(inputs["x"], np.float32)
    NCH = T // CH
    NSLOT = NCH + 1

    def t16(a):
        return np.asarray(a, np.float32).T.copy()

    def padT(a):  # [H, d] -> [512, H] transposed + zero-padded
        t = t16(a)
        out = np.zeros((512, t.shape[1]), np.float32)
        out[:t.shape[0]] = t
        return out

    def browf(bi, bh):
        return np.ascontiguousarray(
            (np.asarray(bi, np.float32) + np.asarray(bh, np.float32))
            .reshape(KH, 128).T)

    wfcT = np.ascontiguousarray(t16(inputs["W_fc"]))
    bfc = np.ascontiguousarray(
        np.asarray(inputs["b_fc"], np.float32).reshape(MO, 128).T)
    shared = {"wfcT": wfcT, "bfc": bfc}

    l0 = {
        "wihT": np.ascontiguousarray(padT(inputs["W_ih0"])),
        "whhT": np.ascontiguousarray(t16(inputs["W_hh0"])),
        "brow": browf(inputs["b_ih0"], inputs["b_hh0"]),
        "msel": np.zeros((128, 1), np.float32),
        "mvec": np.broadcast_to(
            np.r_[0.0, np.ones(NSLOT, np.float32)], (128, NSLOT + 1)).copy(),
    }
    l1 = {
        "wihT": np.ascontiguousarray(padT(inputs["W_ih1"])),
        "whhT": np.ascontiguousarray(t16(inputs["W_hh1"])),
        "brow": browf(inputs["b_ih1"], inputs["b_hh1"]),
        "msel": np.ones((128, 1), np.float32),
        "mvec": np.broadcast_to(
            np.r_[0.0, 0.0, np.ones(NSLOT - 1, np.float32)],
            (128, NSLOT + 1)).copy(),
    }

    zero_xu = np.zeros((2 * 128, NCH * NB), np.float16)
    in_maps = [None] * N_CORES
    for i in range(NPAIR):
        xs = x[i * BCp:(i + 1) * BCp, :T]  # [16, T, D]
        # xu[k, p, s*NB + t*BCp + b] = x[b, s*CH+t, k*128+p]
        xu = np.ascontiguousarray(
            xs.transpose(2, 1, 0).astype(np.float16)
            .reshape(2, 128, NCH, CH, BCp).reshape(2 * 128, NCH * NB))
        in_maps[i] = {"xu": xu, **l0, **shared}
        in_maps[i + NPAIR] = {"xu": zero_xu, **l1, **shared}
    return in_maps


def assemble_out(results):
    out = np.empty((B, O), np.float32)
    for i in range(NPAIR):
        out[i * BCp:(i + 1) * BCp] = results[NPAIR + i]["out"].T
    return out


_NC_CACHE = {}


def kernel(**inputs) -> np.ndarray:
    if "nc" not in _NC_CACHE:
        _NC_CACHE["nc"] = build()
    nc = _NC_CACHE["nc"]
    in_maps = make_in_maps(inputs)
    res = bass_utils.run_bass_kernel_spmd(nc, in_maps, list(range(N_CORES)))
    return assemble_out(res.results)


# revision 9
# speedup vs baseline: 1.5993x; 1.0027x over previous
"""Layer-pipelined Trainium2 Bass kernel for the 2-layer tanh RNN + FC.

Topology: 4 core pairs {i, i+4}, each owning 16 batch samples. Low
cores run layer 0, high cores layer 1, with a one-chunk pipeline lag.
All 8 cores execute the SAME instruction stream (SPMD, no branching);
role differences are carried entirely by per-core input data:

  - uin:   low cores = x chunks (D zero-padded 256->512), high = zeros
  - usb  = uin[chunk s] + msel * AllGather(prev chunk of partner)
           (msel = 0 on low cores, 1 on high cores)
  - whh/wih/brow = the core's layer weights (wih0 zero-padded to 512)
  - maskvec[s] multiplies the hidden state carried across the chunk
    boundary (0 exactly where each role's recurrence must restart)

Per step each core runs only ONE layer's chain: 4 fp32 matmuls
(fp32 operands make each matmul a single self-loading PE instruction
-- fp16 pays a separate LDWEIGHTS per matmul), 1 ScalarE copy, 1
DMA-xbar transpose, 1 DVE add, 1 ScalarE tanh -- roughly half the
fused-two-layer instruction stream. The chunk handoff converts the
fp32 hidden chunk to fp16 once (bulk DVE copy) so the AllGather and
the projection inputs stay 2-byte.
"""

import sys

if "/opt/trn_rl_repo" not in sys.path:
    sys.path.insert(0, "/opt/trn_rl_repo")

import numpy as np

import concourse.bacc as bacc
import concourse.mybir as mybir
import concourse.tile as tile
from concourse import bass_utils

F16 = mybir.dt.float16
F32 = mybir.dt.float32
AF = mybir.ActivationFunctionType

N_CORES = 8
NPAIR = 4
B, T, D, H, O = 64, 1024, 256, 512, 256
BCp = B // NPAIR  # batch per pair: 16

CH = 128  # timesteps per chunk
KH = H // 128  # 4 hidden chunks
KU = 4  # unified input-contraction chunks (D padded to 512)
MO = O // 128  # 2
NB = CH * BCp  # 2048 (t, b) columns per chunk
GROUPS = [[0, 4], [1, 5], [2, 6], [3, 7]]


def build(T=T, CH=CH, reps=1):
    NCH = T // CH
    NSLOT = NCH + 1
    NS = NB // 512  # 512-col moving splits per chunk: 4

    nc = bacc.Bacc("TRN2", target_bir_lowering=False, debug=False,
                   num_devices=N_CORES)

    xu_d = nc.dram_tensor("xu", [2 * 128, NCH * NB], F16,
                          kind="ExternalInput")
    wih_d = nc.dram_tensor("wihT", [512, H], F32, kind="ExternalInput")
    whh_d = nc.dram_tensor("whhT", [H, H], F32, kind="ExternalInput")
    wfc_d = nc.dram_tensor("wfcT", [H, O], F32, kind="ExternalInput")
    brow_d = nc.dram_tensor("brow", [128, KH], F32, kind="ExternalInput")
    bfc_d = nc.dram_tensor("bfc", [128, MO], F32, kind="ExternalInput")
    msel_d = nc.dram_tensor("msel", [128, 1], F32, kind="ExternalInput")
    mvec_d = nc.dram_tensor("mvec", [128, NSLOT + 1], F32,
                            kind="ExternalInput")
    ccin_d = nc.dram_tensor("ccin", [KH * 128, NB], F16, kind="Internal")
    ccout_d = nc.dram_tensor("ccout", [2 * KH * 128, NB], F16,
                             kind="Internal")
    out_d = nc.dram_tensor("out", [O, BCp], F32, kind="ExternalOutput")

    with tile.TileContext(nc) as tc:
        with (
            tc.tile_pool(name="wpool", bufs=1) as wpool,
            tc.tile_pool(name="upool", bufs=1) as upool,
            tc.tile_pool(name="chunks", bufs=2) as chpool,
            tc.tile_pool(name="hprevs", bufs=2) as hpool,
            tc.tile_pool(name="hcts", bufs=1) as hctpool,
            tc.tile_pool(name="state", bufs=4) as spool,
            tc.tile_pool(name="psx", bufs=2, space="PSUM") as psx_pool,
            tc.tile_pool(name="psz", bufs=2, space="PSUM") as psz_pool,
        ):
            # ---- weight/bias preload ----
            wih = wpool.tile([128, KU * KH * 128], F32, name="wih")
            for k in range(KU):
                nc.sync.dma_start(wih[:, k * KH * 128:(k + 1) * KH * 128],
                                  wih_d[k * 128:(k + 1) * 128, :])
            whh = wpool.tile([128, KH * 512], F32, name="whh")
            for k in range(KH):
                nc.sync.dma_start(whh[:, k * 512:(k + 1) * 512],
                                  whh_d[k * 128:(k + 1) * 128, :])
            wfc = wpool.tile([128, KH * MO * 128], F32, name="wfc")
            for k in range(KH):
                nc.sync.dma_start(wfc[:, k * MO * 128:(k + 1) * MO * 128],
                                  wfc_d[k * 128:(k + 1) * 128, :])
            brow = wpool.tile([128, KH], F32, name="brow")
            nc.sync.dma_start(brow[:], brow_d[:])
            bfc = wpool.tile([128, MO], F32, name="bfcs")
            nc.sync.dma_start(bfc[:], bfc_d[:])
            msel = wpool.tile([128, 1], F32, name="msel")
            nc.sync.dma_start(msel[:], msel_d[:])
            mvec = wpool.tile([128, NSLOT + 1], F32, name="mvec")
            nc.sync.dma_start(mvec[:], mvec_d[:])
            hz = wpool.tile([128, KH, BCp], F32, name="hzero")
            nc.vector.memset(hz[:], 0.0)
            zero2k = wpool.tile([128, NB], F16, name="zero2k")
            nc.vector.memset(zero2k[:], 0.0)

            def wsl(wt, k, m, mt):
                c = (k * mt + m) * 128
                return wt[:, c:c + 128]

            for _rep in range(reps):
                # ccout must be zero before slot 0 reads it
                for r in range(2 * KH):
                    nc.sync.dma_start(ccout_d[r * 128:(r + 1) * 128, :],
                                      zero2k[:])
                hprev = hz
                hct = None
                for s in range(NSLOT):
                    # ---- uniform input select:
                    #   usb = xu[chunk s] (D-padded) + msel * ccout[slot 0]
                    usb_x = upool.tile([128, KU, NB], F16, name="usbx")
                    sx = min(s, NCH - 1)  # slot NCH reuses a stale chunk
                    for k in range(2):
                        nc.sync.dma_start(
                            usb_x[:, k, :],
                            xu_d[k * 128:(k + 1) * 128,
                                 sx * NB:(sx + 1) * NB])
                    nc.vector.memset(usb_x[:, 2:4, :], 0.0)
                    usb_cc = upool.tile([128, KU, NB], F16, name="usbcc")
                    for k in range(KU):
                        nc.sync.dma_start(usb_cc[:, k, :],
                                          ccout_d[k * 128:(k + 1) * 128, :])
                    usb_m = upool.tile([128, KU, NB], F16, name="usbm")
                    nc.vector.tensor_scalar_mul(usb_m[:], usb_cc[:],
                                                msel[:, 0:1])
                    usb = upool.tile([128, KU, NB], F32, name="usb")
                    nc.vector.tensor_add(usb[:], usb_x[:], usb_m[:])

                    # ---- projection: xwc[p, t, m, b] ----
                    xwc = chpool.tile([128, CH, KH, BCp], F16, name="xwc")
                    for m in range(KH):
                        for h2 in range(NS // 2):
                            ps = psx_pool.tile([128, 1024], F32, name="psxt")
                            for n2 in range(2):
                                n = h2 * 2 + n2
                                for k in range(KU):
                                    nc.tensor.matmul(
                                        ps[:, n2 * 512:(n2 + 1) * 512],
                                        wsl(wih, k, m, KH),
                                        usb[:, k, n * 512:(n + 1) * 512],
                                        start=(k == 0),
                                        stop=(k == KU - 1),
                                    )
                            tpn = 1024 // BCp  # timesteps per evac: 64
                            nc.scalar.activation(
                                xwc[:, h2 * tpn:(h2 + 1) * tpn, m, :],
                                ps[:].rearrange("p (t b) -> p t b", b=BCp),
                                AF.Identity,
                                bias=brow[:, m:m + 1],
                            )

                    # ---- recurrence (one layer, 128 steps) ----
                    hct = hctpool.tile([128, CH, KH, BCp], F32, name="hct")
                    for tl in range(CH):
                        hsrc = hprev if tl == 0 else hct[:, tl - 1]
                        psz = psz_pool.tile([BCp, 512], F32, name="z")
                        for j in range(KH):
                            nc.tensor.matmul(
                                psz[:],
                                hsrc[:, j, :],
                                whh[:, j * 512:(j + 1) * 512],
                                start=(j == 0),
                                stop=(j == KH - 1),
                            )
                        zsb = spool.tile([BCp, 512], F16, name="zsb")
                        nc.scalar.activation(zsb[:], psz[:], AF.Identity)
                        zt = spool.tile([128, KH, BCp], F16, name="zt")
                        nc.sync.dma_start(zt[:], zsb[:], transpose=True)
                        zpre = spool.tile([128, KH, BCp], F16, name="zpre")
                        nc.vector.tensor_add(zpre[:], zt[:], xwc[:, tl])
                        nc.scalar.activation(hct[:, tl], zpre[:], AF.Tanh)

                    # ---- handoff: send hct (as fp16), gather in pair ----
                    if s < NSLOT - 1:
                        hct16 = hctpool.tile([128, CH, KH, BCp], F16,
                                             name="hct16")
                        nc.vector.tensor_copy(hct16[:], hct[:])
                        for j in range(KH):
                            nc.sync.dma_start(
                                ccin_d[j * 128:(j + 1) * 128, :],
                                hct16[:, :, j, :],
                            )
                        nc.gpsimd.collective_compute(
                            "AllGather",
                            mybir.AluOpType.bypass,
                            ins=[ccin_d[:]],
                            outs=[ccout_d[:]],
                            replica_groups=GROUPS,
                        )
                        # ---- boundary state for the next chunk ----
                        hprev = hpool.tile([128, KH, BCp], F32, name="hprev")
                        nc.vector.tensor_scalar_mul(
                            hprev[:], hct[:, CH - 1], mvec[:, s + 1:s + 2])

            # ---- final FC on hct[:, CH-1] ----
            h1f = hct[:, CH - 1]
            psf = psx_pool.tile([128, MO * BCp], F32, name="psxt")
            for m in range(MO):
                for k in range(KH):
                    nc.tensor.matmul(
                        psf[:, m * BCp:(m + 1) * BCp],
                        wsl(wfc, k, m, MO),
                        h1f[:, k, :],
                        start=(k == 0),
                        stop=(k == KH - 1),
                    )
            outs = spool.tile([128, MO * BCp], F32, name="outs")
            for m in range(MO):
                nc.scalar.activation(
                    outs[:, m * BCp:(m + 1) * BCp],
                    psf[:, m * BCp:(m + 1) * BCp],
                    AF.Identity,
                    bias=bfc[:, m:m + 1],
                )
            for m in range(MO):
                nc.sync.dma_start(out_d[m * 128:(m + 1) * 128, :],
                                  outs[:, m * BCp:(m + 1) * BCp])

    nc.compile()
    return nc


def make_in_maps(inputs, T=T):
    x = np.asarray(inputs["x"], np.float32)
    NCH = T // CH
    NSLOT = NCH + 1

    def t16(a):
        return np.asarray(a, np.float32).T.copy()

    def padT(a):  # [H, d] -> [512, H] transposed + zero-padded
        t = t16(a)
        out = np.zeros((512, t.shape[1]), np.float32)
        out[:t.shape[0]] = t
        return out

    def browf(bi, bh):
        return np.ascontiguousarray(
            (np.asarray(bi, np.float32) + np.asarray(bh, np.float32))
            .reshape(KH, 128).T)

    wfcT = np.ascontiguousarray(t16(inputs["W_fc"]))
    bfc = np.ascontiguousarray(
        np.asarray(inputs["b_fc"], np.float32).reshape(MO, 128).T)
    shared = {"wfcT": wfcT, "bfc": bfc}

    l0 = {
        "wihT": np.ascontiguousarray(padT(inputs["W_ih0"])),
        "whhT": np.ascontiguousarray(t16(inputs["W_hh0"])),
        "brow": browf(inputs["b_ih0"], inputs["b_hh0"]),
        "msel": np.zeros((128, 1), np.float32),
        "mvec": np.broadcast_to(
            np.r_[0.0, np.ones(NSLOT, np.float32)], (128, NSLOT + 1)).copy(),
    }
    l1 = {
        "wihT": np.ascontiguousarray(padT(inputs["W_ih1"])),
        "whhT": np.ascontiguousarray(t16(inputs["W_hh1"])),
        "brow": browf(inputs["b_ih1"], inputs["b_hh1"]),
        "msel": np.ones((128, 1), np.float32),
        "mvec": np.broadcast_to(
            np.r_[0.0, 0.0, np.ones(NSLOT - 1, np.float32)],
            (128, NSLOT + 1)).copy(),
    }

    zero_xu = np.zeros((2 * 128, NCH * NB), np.float16)
    in_maps = [None] * N_CORES
    for i in range(NPAIR):
        xs = x[i * BCp:(i + 1) * BCp, :T]  # [16, T, D]
        # xu[k, p, s*NB + t*BCp + b] = x[b, s*CH+t, k*128+p]
        xu = np.ascontiguousarray(
            xs.transpose(2, 1, 0).astype(np.float16)
            .reshape(2, 128, NCH, CH, BCp).reshape(2 * 128, NCH * NB))
        in_maps[i] = {"xu": xu, **l0, **shared}
        in_maps[i + NPAIR] = {"xu": zero_xu, **l1, **shared}
    return in_maps


def assemble_out(results):
    out = np.empty((B, O), np.float32)
    for i in range(NPAIR):
        out[i * BCp:(i + 1) * BCp] = results[NPAIR + i]["out"].T
    return out


_NC_CACHE = {}


def kernel(**inputs) -> np.ndarray:
    if "nc" not in _NC_CACHE:
        _NC_CACHE["nc"] = build()
    nc = _NC_CACHE["nc"]
    in_maps = make_in_maps(inputs)
    res = bass_utils.run_bass_kernel_spmd(nc, in_maps, list(range(N_CORES)))
    return assemble_out(res.results)
